# revision 30
# baseline (speedup 1.0000x reference)
"""Trainium2 Bass kernel for nn_MirasModel (scatter_memory).

Strategy (8 NeuronCores, SPMD):
  - Column-shard the shared D=3136 feature dimension: core c owns Dc=392
    columns of dense_k_w / dense_v_w / mem_w2 / biases / scales, and the
    matching 392 rows of mem_w1.
  - Conv + rmsnorm computed fully on every core (tiny) via a packed
    im2col matmul, with a DMA scatter producing the transposed
    [Din, T] activation layout the dense matmuls need.
  - Three AllReduce rounds:
      R1: z1 = keys@w1+b1 partial sums  +  Gram(keys) = keys keys^T
      R2: per-token scalars (C,A,B) + backward projections P1,P2,P3
      R3: final-forward rmsnorm scalar partials
    The Gram matrix lets z1f = z1 - G_K diag(w) dz1 be computed locally,
    eliminating a fourth round (keys @ agg_w1 == Gram @ diag(w) @ dz1).
  - All heavy DMA (im2col + dense weight shards) hides under R1's
    collective entry latency.
"""

import sys

if '/opt/trn_rl_repo' not in sys.path:
    sys.path.insert(0, '/opt/trn_rl_repo')

import numpy as np

import concourse.bass as bass
import concourse.mybir as mybir
from concourse import tile
from concourse.bass_utils import run_bass_kernel_spmd

F32 = mybir.dt.float32
F32R = mybir.dt.float32r
AF = mybir.ActivationFunctionType
OP = mybir.AluOpType

T = 64
D = 3136
H = 512
NCORES = 8
DC = D // NCORES            # 392 columns per core
CQ = 98                     # Dc sub-chunk (4 per core)
NQ = DC // CQ               # 4
PPIX = 800                  # padded pixel count (784 real + 16 dummy)
DINP = PPIX * 4             # padded Din = 3200
RT = DINP // 128            # 25 Din tiles
NPTR = PPIX // 2            # 400 pixel-pairs
NCONV = NPTR * T // 512     # 50 conv matmul chunks
HT = H // 128               # 4 H tiles
ALPHA, ETA0, EPS = 0.9, 0.1, 1e-6

_NC_CACHE = {}


# ---------------------------------------------------------------------------
# walrus workaround: this compiler build rejects Drain instructions carrying
# more than one sync wait; split extras onto preceding Drains.
def _split_excess_waits(nc):
    """This walrus build has tight per-instruction sync-wait budgets
    (1 for Drain/Matmult/etc).  Move excess waits onto preceding NoOps."""
    LIM1 = 1

    def limit_for(ins):
        return LIM1

    n_new = 0
    for fn in nc.m.functions:
        for bb in fn.blocks:
            i = 0
            while i < len(bb.instructions):
                ins = bb.instructions[i]
                si = getattr(ins, 'sync_info', None)
                lim = limit_for(ins)
                if (si is not None and si.on_wait and len(si.on_wait) > lim
                        and getattr(ins, 'engine', None) is not None):
                    waits = list(si.on_wait)
                    keep, extra = waits[:lim], waits[lim:]
                    ins.sync_info = mybir.SyncInfo(on_wait=keep,
                                                  on_update=si.on_update)
                    pos = i
                    for j in range(0, len(extra), LIM1):
                        n_new += 1
                        nd = mybir.InstNoOp(
                            name=f"I-waitfix-{n_new}",
                            engine=ins.engine,
                            bass_nofuse=True,
                            sync_info=mybir.SyncInfo(
                                on_wait=extra[j:j + LIM1], on_update=[]),
                        )
                        bb.instructions.insert(pos, nd)
                        pos += 1
                        i += 1
                i += 1
    return n_new


def _din_perm():
    """Device Din row -> reference Din index (p*4+c), p,c of padded grid."""
    idx = np.zeros(DINP, np.int64)
    valid = np.zeros(DINP, bool)
    for r in range(RT):
        for i in range(128):
            g, c, jj = i // 64, (i % 64) // 16, i % 16
            p = 2 * (16 * r + jj) + g
            row = r * 128 + i
            if p < 784:
                idx[row] = p * 4 + c
                valid[row] = True
    return idx, valid


def _build_im2col(x_t, pad_val=0.0):
    """x_t: (T, 28, 28, 4) NHWC.  Returns X72 [73, NPTR*64] fp32.

    row = g*36 + (di*3+dj)*4 + ci  (g in 0..1), row 72 = ones.
    col = ptr*64 + t, pixel p = 2*ptr + g (row-major 28x28, padded to 800).
    """
    xp = np.zeros((T, 30, 30, 4), np.float32)
    xp[:, 1:29, 1:29, :] = x_t
    X = np.zeros((73, NPTR * T), np.float32)
    p = np.arange(PPIX)
    pi, pj = p // 28, p % 28
    ok = p < 784
    for g in range(2):
        psel = p[(p % 2) == g]
        ptr = psel // 2
        pis, pjs, oks = pi[(p % 2) == g], pj[(p % 2) == g], ok[(p % 2) == g]
        for di in range(3):
            for dj in range(3):
                for ci in range(4):
                    row = g * 36 + (di * 3 + dj) * 4 + ci
                    vals = np.zeros((NPTR, T), np.float32)
                    vsel = xp[:, np.clip(pis + di, 0, 29),
                              np.clip(pjs + dj, 0, 29), ci]  # (T, NPTR)
                    vals[oks[: NPTR], :] = vsel.T[oks[: NPTR], :]
                    # dummy pixels (>=784) contribute garbage later discarded
                    X[row, :] = vals.reshape(-1)
    X[72, :] = 1.0
    return X


def _build_w73(conv_k_w, conv_k_b, conv_v_w, conv_v_b):
    """W73 [73, 16]; col = g*8 + kv*4 + co."""
    W = np.zeros((73, 16), np.float32)
    for g in range(2):
        for kv, (w, b) in enumerate(((conv_k_w, conv_k_b),
                                     (conv_v_w, conv_v_b))):
            for di in range(3):
                for dj in range(3):
                    for ci in range(4):
                        W[g * 36 + (di * 3 + dj) * 4 + ci,
                          g * 8 + kv * 4:g * 8 + kv * 4 + 4] = w[di, dj, ci, :]
            W[72, g * 8 + kv * 4:g * 8 + kv * 4 + 4] = b
    return W


def _rms_pattern(scale4):
    """[128,1] per-partition rms scale: partition i -> scale4[(i%64)//16]."""
    i = np.arange(128)
    return scale4[(i % 64) // 16].astype(np.float32).reshape(128, 1)


def _s4():
    """S4 [128, 32]: S4[i, p32] = 1 iff p32 == (i//64)*16 + (i%16)."""
    S = np.zeros((128, 32), np.float32)
    i = np.arange(128)
    S[i, (i // 64) * 16 + (i % 16)] = 1.0
    return S


def _wvec():
    betas = (np.float32(ALPHA) ** np.arange(T, dtype=np.float32)).astype(np.float32)
    etas = (np.float32(ETA0) * betas).astype(np.float32)
    weights = (etas * (betas[-1] / betas)).astype(np.float32)
    return (np.float32(1e-4) * weights).astype(np.float32)


def build_nc(debug=False):
    nc = bass.Bass()

    def inp(name, shape, dt=F32):
        return nc.dram_tensor(name, list(shape), dt, kind="ExternalInput")

    # chunk-contiguous im2col padded to 80 rows: the HWDGE only fans a
    # transfer across the 16 SDMA engines when the partition count splits
    # evenly; 73 rows pinned every packet to one engine (~24 GB/s).
    X72 = inp('X72', (NCONV * 80, 512), F32R)
    W73 = inp('W73', (73, 16), F32R)
    DCP = 512               # padded per-core columns (2x256 f32r)
    WkC = inp('WkC', (DINP, DCP), F32R)
    WvC = inp('WvC', (DINP, DCP), F32R)
    bkC = inp('bkC', (1, DCP), F32R)
    bvC = inp('bvC', (1, DCP), F32R)
    w1C = inp('w1C', (CQ, NQ * H))   # w1 rows chunked: [:, q*H+h]
    b1row8 = inp('b1row8', (1, H))        # mem_b1 / 8
    w2C = inp('w2C', (128, HT * DC))  # w2 H-chunked: [:, m*DC+d]
    b2C = inp('b2C', (1, DC))
    scC = inp('scC', (1, DC))
    rosC = inp('rosC', (1, DC))
    scsqT = inp('scsqT', (CQ, NQ))        # mem_scale[sl]**2, chunked columns
    rmspk = inp('rmspk', (128, 1))
    rmspv = inp('rmspv', (128, 1))
    S4 = inp('S4', (128, 32), F32R)
    wv = inp('wv', (T, 1))                # 1e-4 * weights
    ones1x64 = inp('ones1x64', (1, T))
    ones1x128 = inp('ones1x128', (1, 128))
    onescol = inp('onescol', (128, 1))
    ident = inp('ident', (128, 128))

    out = nc.dram_tensor('out', [DC, T], F32, kind="ExternalOutput")
    dbg_outs = {}

    def dbg(name, shape):
        if debug:
            dbg_outs[name] = nc.dram_tensor(name, list(shape), F32,
                                            kind="ExternalOutput")
        return dbg_outs.get(name)

    d_nkT = dbg('d_nkT', (128, RT * T))
    d_keys = dbg('d_keys', (T, DC))
    d_vals = dbg('d_vals', (T, DC))
    d_z1T = dbg('d_z1T', (H, T))
    d_GK = dbg('d_GK', (T, T))
    d_y = dbg('d_y', (T, DC))
    d_P = dbg('d_P', (3 * H, T))
    d_dhT = dbg('d_dhT', (H, T))
    d_z1fT = dbg('d_z1fT', (H, T))
    d_w2p = dbg('d_w2p', (H, DC))
    d_yfT = dbg('d_yfT', (DC, T))

    with tile.TileContext(nc) as tc:
        with (
            tc.tile_pool(name='consts', bufs=1) as pc,
            tc.tile_pool(name='wstream', bufs=4) as pw,
            tc.tile_pool(name='xstream', bufs=4) as px,
            tc.tile_pool(name='big', bufs=1) as pb,
            tc.tile_pool(name='work', bufs=1) as pk,
            tc.tile_pool(name='psA', bufs=2, space='PSUM') as psA,
            tc.tile_pool(name='psB', bufs=2, space='PSUM') as psB,
            tc.tile_pool(name='dram', bufs=1, space='DRAM') as pd,
        ):
            # ---- small constants to SBUF ----
            def lc(ap, shape, name, dt=F32):
                t_ = pc.tile(list(shape), dt, name=name)
                nc.sync.dma_start(t_[:], ap[:])
                return t_

            W73s = lc(W73, (73, 16), 'W73s', F32R)
            o64r = pc.tile([1, T], F32R, name='o64r')
            nc.sync.dma_start(o64r[:], ones1x64[:].bitcast(F32R))
            bkS = lc(bkC, (1, DCP), 'bkS', F32R)
            bvS = lc(bvC, (1, DCP), 'bvS', F32R)
            w1S = lc(w1C, (CQ, NQ * H), 'w1S')
            b1r8 = lc(b1row8, (1, H), 'b1r8')
            w2S = lc(w2C, (128, HT * DC), 'w2S')
            b2S = lc(b2C, (1, DC), 'b2S')
            scS = lc(scC, (1, DC), 'scS')
            rosS = lc(rosC, (1, DC), 'rosS')
            scsqTS = lc(scsqT, (CQ, NQ), 'scsqTS')
            rpk = lc(rmspk, (128, 1), 'rpk')
            rpv = lc(rmspv, (128, 1), 'rpv')
            S4s = lc(S4, (128, 32), 'S4s', F32R)
            wvS = lc(wv, (T, 1), 'wvS')
            o64 = lc(ones1x64, (1, T), 'o64')
            o128 = lc(ones1x128, (1, 128), 'o128')
            ocol = lc(onescol, (128, 1), 'ocol')
            idn = lc(ident, (128, 128), 'idn')
            epsT = pc.tile([128, 1], F32, name='epsT')
            nc.gpsimd.memset(epsT[:], EPS)

            def r_(ap):
                return ap.bitcast(F32R)

            # =========== PHASE 1 ===========
            # conv: 50 chunks of 512 cols; output rows 16 = (g, kv, c)
            # staged in 5 groups of 10 chunks to bound SBUF usage
            convT = {0: pb.tile([128, RT * T], F32, name='convT0'),
                     1: pb.tile([128, RT * T], F32, name='convT1')}
            Dscr = pd.tile([16 * 16, RT * T], F32, name='Dscr')
            DscrW = Dscr[:].rearrange('(p j) f -> p j f', p=16)
            DscrR = Dscr[:].rearrange('(g k cj) f -> g k cj f', g=2, k=2)
            RG = 5                      # r-tiles per scatter group
            for gi in range(RT // RG):
                # cg free layout: (j:16, r:RG, t:64) so scatter src is contiguous
                cg = px.tile([16, 16 * RG * T], F32, name='cg', tag='cg',
                             bufs=2)
                cg4 = cg[:].rearrange('p (j r t) -> p j r t', j=16, r=RG)
                for ni in range(2 * RG):
                    n = gi * 2 * RG + ni
                    half, rl = ni % 2, ni // 2
                    xt = px.tile([80, 512], F32R, name='xch', tag='xch')
                    nc.sync.dma_start(xt[:], X72[n * 80:(n + 1) * 80, :])
                    ps = psA.tile([16, 512], F32, name='cps', tag='cps')
                    nc.tensor.matmul(ps[:], W73s[:], xt[0:73, :],
                                     start=True, stop=True)
                    ps3 = ps[:].rearrange('p (j t) -> p j t', j=8)
                    dst3 = cg4[:, half * 8:(half + 1) * 8, rl, :]
                    if ni % 2 == 0:
                        nc.scalar.activation(dst3, ps3, AF.Copy)
                    else:
                        nc.vector.tensor_copy(dst3, ps3)
                # spill group to DRAM scratch, row = (g,kv,c,j); HWDGE
                # generates the 256 descriptors in hardware
                nc.scalar.dma_start(
                    DscrW[:, :, gi * RG * T:(gi + 1) * RG * T], cg[:])
            # gather back: per kv one read of 128 contiguous 6400B runs
            for kv in range(2):
                nc.scalar.dma_start(convT[kv][:], DscrR[:, kv])

            # rmsnorm: squares, per-pixel sumsq, rsqrt, dup-scatter, scale
            nkT = {}
            for kv in range(2):
                sqall = px.tile([128, RT * T], F32R, name='sqall', tag='sqall',
                                bufs=1)
                nc.scalar.activation(sqall[:], convT[kv][:], AF.Square)
                ssall = px.tile([32, RT * T], F32, name='ssall', tag='ssall',
                                bufs=1)
                for rb in range((RT + 3) // 4):
                    r0 = rb * 4
                    nr = min(4, RT - r0)
                    sl = slice(r0 * T, (r0 + nr) * T)
                    ss = psB.tile([32, 4 * T], F32, name='ssq', tag='mm64')
                    nc.tensor.matmul(ss[:, 0:nr * T], S4s[:],
                                     sqall[:, sl], start=True, stop=True)
                    nc.vector.tensor_scalar(ssall[:, sl], ss[:, 0:nr * T],
                                            0.25, EPS, OP.mult, OP.add)
                inv32 = pb.tile([32, RT * T], F32, name=f'inv32_{kv}',
                                tag='inv32', bufs=2)
                nc.scalar.activation(ssall[:], ssall[:], AF.Ln)
                nc.scalar.activation(inv32[:], ssall[:], AF.Exp, scale=-0.5)
                invP = pb.tile([128, RT * T], F32, name=f'invP{kv}',
                               tag='invP', bufs=1)
                for g in range(2):
                    for c in range(4):
                        dst = invP[:].rearrange('(g c j) f -> g c j f',
                                                g=2, c=4)
                        src = inv32[:].rearrange('(g j) f -> g j f', g=2)
                        nc.gpsimd.dma_start(dst[g, c], src[g])
                nT = pb.tile([128, RT * T], F32R, name=f'nkT{kv}')
                nkT[kv] = nT
                rp = rpk if kv == 0 else rpv
                nc.vector.scalar_tensor_tensor(
                    nT[:], convT[kv][:], rp[:], invP[:],
                    OP.mult, OP.mult)
            if debug:
                nc.sync.dma_start(d_nkT[:], nkT[0][:])

            # dense: keys/vals [T, DC] (T on partitions)
            kv_sb = {}
            for kv, (Wap, bS) in enumerate(((WkC, bkS), (WvC, bvS))):
                ps = psA.tile([T, DCP], F32, name='dps', tag='dps')
                for r in range(RT):
                    wt = pw.tile([128, DCP], F32R, name='wt', tag='wt', bufs=5)
                    nc.sync.dma_start(wt[:], Wap[r * 128:(r + 1) * 128, :])
                    nc.tensor.matmul(ps[:],
                                     nkT[kv][:, r * T:(r + 1) * T],
                                     wt[:],
                                     start=(r == 0), stop=False)
                nc.tensor.matmul(ps[:], o64r[:],
                                 bS[:], start=False, stop=True)
                sb = pk.tile([T, DC], F32, name=f'kv{kv}')
                nc.vector.tensor_copy(sb[:], ps[:, 0:DC])
                kv_sb[kv] = sb
            keys, vals = kv_sb[0], kv_sb[1]
            if debug:
                nc.sync.dma_start(d_keys[:], keys[:])
                nc.sync.dma_start(d_vals[:], vals[:])

            # transpose keys -> keysT chunks [98, 64] x4
            keysT = pk.tile([CQ, NQ * T], F32, name='keysT')
            for q in range(NQ):
                pt = psB.tile([CQ, T], F32, name='tps', tag='mm64')
                nc.tensor.transpose(pt[:], keys[:, q * CQ:(q + 1) * CQ],
                                    idn[0:T, 0:T])
                nc.vector.tensor_copy(keysT[:, q * T:(q + 1) * T], pt[:])

            # scb = bcast(sc), scb2, q2 = vals*scb, scv = scb*vals transposed
            psc = psA.tile([T, DC], F32, name='pscb', tag='dps')
            nc.tensor.matmul(psc[:], o64[:], scS[:],
                             start=True, stop=True)
            scb = pk.tile([T, DC], F32, name='scb')
            nc.vector.tensor_copy(scb[:], psc[:])
            scb2 = pk.tile([T, DC], F32, name='scb2')
            nc.vector.tensor_tensor(scb2[:], scb[:], scb[:], OP.mult)
            q2 = pk.tile([T, DC], F32, name='q2')
            nc.vector.tensor_tensor(q2[:], vals[:], scb[:], OP.mult)
            scvT = pk.tile([CQ, NQ * T], F32, name='scvT')
            for q in range(NQ):
                pt = psB.tile([CQ, T], F32, name='tps', tag='mm64')
                nc.tensor.transpose(pt[:], q2[:, q * CQ:(q + 1) * CQ],
                                    idn[0:T, 0:T])
                nc.vector.tensor_copy(scvT[:, q * T:(q + 1) * T], pt[:])

            # w2T chunks [98, 512] x4 (PE transposes)
            w2T = pk.tile([CQ, NQ * H], F32, name='w2T')
            for q in range(NQ):
                for m in range(HT):
                    pt = psB.tile([CQ, 128], F32, name='t2ps', tag='mm64')
                    nc.tensor.transpose(
                        pt[:], w2S[:, m * DC + q * CQ:
                                   m * DC + (q + 1) * CQ], idn[:])
                    nc.vector.tensor_copy(
                        w2T[:, q * H + m * 128:q * H + (m + 1) * 128], pt[:])

            # G_K = keys keys^T  (accumulate over chunks)
            pgk = psB.tile([T, T], F32, name='pgk', tag='acc')
            for q in range(NQ):
                nc.tensor.matmul(pgk[:], keysT[:, q * T:(q + 1) * T],
                                 keysT[:, q * T:(q + 1) * T],
                                 start=(q == 0), stop=(q == NQ - 1))
            GK = pk.tile([T, T], F32, name='GK')
            nc.vector.tensor_copy(GK[:], pgk[:])
            if debug:
                nc.sync.dma_start(d_GK[:], GK[:])

            # z1T partial [H(4x128), T] = w1C^T keysT + b1/8
            z1Tp = pk.tile([128, HT * T], F32, name='z1Tp')
            for m in range(HT):
                pz = psB.tile([128, T], F32, name='pz', tag='acc')
                for q in range(NQ):
                    nc.tensor.matmul(pz[:],
                                     w1S[:, q * H + m * 128:
                                         q * H + (m + 1) * 128],
                                     keysT[:, q * T:(q + 1) * T],
                                     start=(q == 0), stop=False)
                nc.tensor.matmul(pz[:], b1r8[:, m * 128:(m + 1) * 128],
                                 o64[:], start=False, stop=True)
                nc.vector.tensor_copy(z1Tp[:, m * T:(m + 1) * T], pz[:])

            # ---- R1: AllReduce [z1T ; GK] ----
            r1i = pd.tile([H + T, T], F32, name='r1i')
            r1o = pd.tile([H + T, T], F32, name='r1o')
            for m in range(HT):
                nc.gpsimd.dma_start(r1i[m * 128:(m + 1) * 128, :],
                                    z1Tp[:, m * T:(m + 1) * T])
            nc.gpsimd.dma_start(r1i[H:H + T, :], GK[:])
            nc.gpsimd.collective_compute(
                'AllReduce', OP.add, replica_groups=[list(range(NCORES))],
                ins=[r1i.opt()], outs=[r1o.opt()])

            z1T = pk.tile([128, HT * T], F32, name='z1T')
            for m in range(HT):
                nc.sync.dma_start(z1T[:, m * T:(m + 1) * T],
                                  r1o[m * 128:(m + 1) * 128, :])
            GKg = pk.tile([T, T], F32, name='GKg')
            nc.sync.dma_start(GKg[:], r1o[H:H + T, :])
            if debug:
                for m in range(HT):
                    nc.sync.dma_start(d_z1T[m * 128:(m + 1) * 128, :],
                                      z1T[:, m * T:(m + 1) * T])

            # R64 = diag(wv) @ (GK + 1)  (for z1f correction incl. agg_b1)
            R64 = pk.tile([T, T], F32, name='R64')
            nc.vector.tensor_scalar(R64[:], GKg[:], 1.0, wvS[:],
                                    OP.add, OP.mult)

            # =========== PHASE 2 ===========
            hT = pk.tile([128, HT * T], F32, name='hT')
            for m in range(HT):
                nc.scalar.activation(hT[:, m * T:(m + 1) * T],
                                     z1T[:, m * T:(m + 1) * T],
                                     AF.Gelu_apprx_tanh)
            # h [T, H]
            h = pk.tile([T, H], F32, name='h')
            for m in range(HT):
                pt = psB.tile([T, 128], F32, name='hps', tag='mm64')
                nc.tensor.transpose(pt[:], hT[:, m * T:(m + 1) * T], idn[:])
                nc.vector.tensor_copy(h[:, m * 128:(m + 1) * 128], pt[:])

            # y = h @ w2C  [T, DC]
            py = psA.tile([T, DC], F32, name='py', tag='dps')
            for m in range(HT):
                nc.tensor.matmul(py[:], hT[:, m * T:(m + 1) * T],
                                 w2S[:, m * DC:(m + 1) * DC],
                                 start=(m == 0), stop=(m == HT - 1))
            y = pk.tile([T, DC], F32, name='y')
            nc.vector.tensor_copy(y[:], py[:])
            if debug:
                nc.sync.dma_start(d_y[:], y[:])

            # yT chunks + (sc^2 y)T
            yT = pk.tile([CQ, NQ * T], F32, name='yT')
            s2yT = pk.tile([CQ, NQ * T], F32, name='s2yT')
            for q in range(NQ):
                pt = psB.tile([CQ, T], F32, name='tps', tag='mm64')
                nc.tensor.transpose(pt[:], y[:, q * CQ:(q + 1) * CQ],
                                    idn[0:T, 0:T])
                nc.vector.tensor_copy(yT[:, q * T:(q + 1) * T], pt[:])
                nc.vector.tensor_scalar(s2yT[:, q * T:(q + 1) * T],
                                        yT[:, q * T:(q + 1) * T],
                                        scsqTS[:, q:q + 1], None,
                                        OP.mult)

            # scalars C = sum y^2, A = sum (scb y)^2, B = sum (scb y) v
            ua = pk.tile([T, DC], F32, name='ua')
            nc.vector.tensor_tensor(ua[:], y[:], scb[:], OP.mult)
            scr = pk.tile([T, DC], F32, name='scr')
            Cc = pk.tile([T, 1], F32, name='Cc')
            Ac = pk.tile([T, 1], F32, name='Ac')
            Bc = pk.tile([T, 1], F32, name='Bc')
            nc.scalar.activation(scr[:], y[:], AF.Square, accum_out=Cc[:])
            nc.scalar.activation(scr[:], ua[:], AF.Square, accum_out=Ac[:])
            nc.vector.scalar_tensor_tensor(scr[:], ua[:], 1.0, vals[:],
                                           OP.mult, OP.mult,
                                           accum_out=Bc[:])

            # ---- R2a: tiny early AllReduce of per-token scalars C,A,B ----
            r2ai = pd.tile([3, T], F32, name='r2ai')
            r2ao = pd.tile([3, T], F32, name='r2ao')
            nc.gpsimd.dma_start(r2ai[0:1, :], Cc[:])
            nc.gpsimd.dma_start(r2ai[1:2, :], Ac[:])
            nc.gpsimd.dma_start(r2ai[2:3, :], Bc[:])
            nc.gpsimd.collective_compute(
                'AllReduce', OP.add, replica_groups=[list(range(NCORES))],
                ins=[r2ai.opt()], outs=[r2ao.opt()])

            # P matmuls (local partials; run while R2a is in flight)
            Pt = pk.tile([128, 3 * HT * T], F32, name='Pt')
            rhs_list = [s2yT, scvT, yT]
            for pi, rhs in enumerate(rhs_list):
                for m in range(HT):
                    pp = psB.tile([128, T], F32, name='pp', tag='acc')
                    for q in range(NQ):
                        nc.tensor.matmul(
                            pp[:],
                            w2T[:, q * H + m * 128:q * H + (m + 1) * 128],
                            rhs[:, q * T:(q + 1) * T],
                            start=(q == 0), stop=(q == NQ - 1))
                    nc.vector.tensor_copy(
                        Pt[:, (pi * HT + m) * T:(pi * HT + m + 1) * T], pp[:])

            Ct = pk.tile([1, T], F32, name='Ct')
            At = pk.tile([1, T], F32, name='At')
            Bt = pk.tile([1, T], F32, name='Bt')
            nc.sync.dma_start(Ct[:], r2ao[0:1, :])
            nc.sync.dma_start(At[:], r2ao[1:2, :])
            nc.sync.dma_start(Bt[:], r2ao[2:3, :])

            # scalar rows (each its own [1,T] tile, base partition 0)
            invt = pk.tile([1, T], F32, name='invt')
            i2t = pk.tile([1, T], F32, name='i2t')
            St = pk.tile([1, T], F32, name='St')
            s2t = pk.tile([1, T], F32, name='s2t')
            a1t = pk.tile([1, T], F32, name='a1t')
            a2t = pk.tile([1, T], F32, name='a2t')
            a3t = pk.tile([1, T], F32, name='a3t')
            nc.scalar.activation(invt[:], Ct[:], AF.Sqrt,
                                 bias=epsT[0:1, :], scale=1.0 / D)
            nc.vector.reciprocal(invt[:], invt[:])
            nc.vector.tensor_tensor(i2t[:], invt[:], invt[:], OP.mult)
            # S = 2 inv A - 2 B
            nc.vector.tensor_tensor(St[:], invt[:], At[:], OP.mult)
            nc.vector.tensor_scalar(St[:], St[:], 2.0, None, OP.mult)
            nc.vector.tensor_scalar(s2t[:], Bt[:], 2.0, None, OP.mult)
            nc.vector.tensor_tensor(St[:], St[:], s2t[:], OP.subtract)
            # a3 = inv^3 S / D ; a1 = 2 inv^2 ; a2 = 2 inv
            nc.vector.tensor_tensor(a3t[:], i2t[:], invt[:], OP.mult)
            nc.vector.tensor_tensor(a3t[:], a3t[:], St[:], OP.mult)
            nc.vector.tensor_scalar(a3t[:], a3t[:], 1.0 / D, None, OP.mult)
            nc.vector.tensor_scalar(a1t[:], i2t[:], 2.0, None, OP.mult)
            nc.vector.tensor_scalar(a2t[:], invt[:], 2.0, None, OP.mult)

            # broadcast a1,a2,a3 to [128, T]
            ab = pk.tile([128, 3 * T], F32, name='ab')
            for j, row in enumerate((a1t, a2t, a3t)):
                pt = psB.tile([128, T], F32, name='abps', tag='mm64')
                nc.tensor.matmul(pt[:], o128[:], row[:], start=True, stop=True)
                nc.vector.tensor_copy(ab[:, j * T:(j + 1) * T], pt[:])

            # column versions: srow4 = [inv; a1; a2; a3] -> scol [T, 4]
            srow4 = pk.tile([4, T], F32, name='srow4')
            for j, row in enumerate((invt, a1t, a2t, a3t)):
                nc.gpsimd.dma_start(srow4[j:j + 1, :], row[:])
            scol = pk.tile([T, 4], F32, name='scol')
            pt = psB.tile([T, 4], F32, name='scps', tag='mm64')
            nc.tensor.transpose(pt[:], srow4[:], idn[0:4, 0:4])
            nc.vector.tensor_copy(scol[:], pt[:])
            # scol cols: 0 inv, 1 a1, 2 a2, 3 a3

            # local combine m = a1*P1 - a2*P2 - a3*P3, then R2b AllReduce [H,T]
            mloc = pk.tile([128, HT * T], F32, name='mloc')
            tmp2 = pk.tile([128, T], F32, name='tmp2')
            for m in range(HT):
                msl = slice(m * T, (m + 1) * T)
                nc.vector.tensor_tensor(mloc[:, msl], Pt[:, msl],
                                        ab[:, 0:T], OP.mult)
                nc.vector.tensor_tensor(tmp2[:],
                                        Pt[:, (HT + m) * T:(HT + m + 1) * T],
                                        ab[:, T:2 * T], OP.mult)
                nc.vector.tensor_tensor(mloc[:, msl], mloc[:, msl], tmp2[:],
                                        OP.subtract)
                nc.vector.tensor_tensor(
                    tmp2[:], Pt[:, (2 * HT + m) * T:(2 * HT + m + 1) * T],
                    ab[:, 2 * T:3 * T], OP.mult)
                nc.vector.tensor_tensor(mloc[:, msl], mloc[:, msl], tmp2[:],
                                        OP.subtract)

            r2i = pd.tile([H, T], F32, name='r2i')
            r2o = pd.tile([H, T], F32, name='r2o')
            for j in range(HT):
                nc.gpsimd.dma_start(r2i[j * 128:(j + 1) * 128, :],
                                    mloc[:, j * T:(j + 1) * T])
            nc.gpsimd.collective_compute(
                'AllReduce', OP.add, replica_groups=[list(range(NCORES))],
                ins=[r2i.opt()], outs=[r2o.opt()])

            # dz1T = dhT_global * gelu'(z1T)
            dz1T = pk.tile([128, HT * T], F32, name='dz1T')
            tmp1 = pk.tile([128, T], F32, name='tmp1')
            for m in range(HT):
                nc.sync.dma_start(tmp1[:], r2o[m * 128:(m + 1) * 128, :])
                nc.scalar.activation(tmp2[:], z1T[:, m * T:(m + 1) * T],
                                     AF.Derivative_Gelu)
                nc.vector.tensor_tensor(dz1T[:, m * T:(m + 1) * T], tmp1[:],
                                        tmp2[:], OP.mult)

            # dz1 [T, H]
            dz1 = pk.tile([T, H], F32, name='dz1')
            for m in range(HT):
                pt = psB.tile([T, 128], F32, name='dzps', tag='mm64')
                nc.tensor.transpose(pt[:], dz1T[:, m * T:(m + 1) * T], idn[:])
                nc.vector.tensor_copy(dz1[:, m * 128:(m + 1) * 128], pt[:])

            # z1fT = z1T - dz1^T-weighted: T2T[m] = dz1[:,m]^T ... via matmul
            z1fT = pk.tile([128, HT * T], F32, name='z1fT')
            hfT = pk.tile([128, HT * T], F32, name='hfT')
            for m in range(HT):
                pt = psB.tile([128, T], F32, name='t2t', tag='mm64')
                nc.tensor.matmul(pt[:], dz1[:, m * 128:(m + 1) * 128],
                                 R64[:], start=True, stop=True)
                nc.vector.tensor_tensor(z1fT[:, m * T:(m + 1) * T],
                                        z1T[:, m * T:(m + 1) * T], pt[:],
                                        OP.subtract)
                if debug:
                    nc.sync.dma_start(d_z1fT[m * 128:(m + 1) * 128, :],
                                      z1fT[:, m * T:(m + 1) * T])
                nc.scalar.activation(hfT[:, m * T:(m + 1) * T],
                                     z1fT[:, m * T:(m + 1) * T],
                                     AF.Gelu_apprx_tanh)

            # G = a1*(scb2*y) - a2*(q2) - a3*y  (column scalars)
            G = pk.tile([T, DC], F32, name='G')
            gt1 = pk.tile([T, DC], F32, name='gt1')
            nc.vector.tensor_tensor(gt1[:], y[:], scb2[:], OP.mult)
            nc.vector.tensor_scalar(G[:], gt1[:], scol[:, 1:2], None, OP.mult)
            nc.vector.tensor_scalar(gt1[:], q2[:], scol[:, 2:3], None, OP.mult)
            nc.vector.tensor_tensor(G[:], G[:], gt1[:], OP.subtract)
            nc.vector.tensor_scalar(gt1[:], y[:], scol[:, 3:4], None, OP.mult)
            nc.vector.tensor_tensor(G[:], G[:], gt1[:], OP.subtract)

            # agg_w2 & w2' = w2 - h^T (wv*G)
            wG = pk.tile([T, DC], F32, name='wG')
            nc.vector.tensor_scalar(wG[:], G[:], wvS[:], None, OP.mult)
            w2p = pk.tile([128, HT * DC], F32, name='w2p')
            for m in range(HT):
                pa = psA.tile([128, DC], F32, name='paw2', tag='dps')
                nc.tensor.matmul(pa[:],
                                 h[:, m * 128:(m + 1) * 128],
                                 wG[:], start=True, stop=True)
                nc.vector.tensor_tensor(w2p[:, m * DC:(m + 1) * DC],
                                        w2S[:, m * DC:(m + 1) * DC], pa[:],
                                        OP.subtract)
                if debug:
                    nc.sync.dma_start(d_w2p[m * 128:(m + 1) * 128, :],
                                      w2p[:, m * DC:(m + 1) * DC])

            # b2' row and sc' row; scros = sc' * ros
            pr = psB.tile([1, DC], F32, name='prow', tag='acc')
            nc.tensor.matmul(pr[:], wvS[:], G[:],
                             start=True, stop=True)
            b2p = pk.tile([1, DC], F32, name='b2p')
            nc.vector.tensor_tensor(b2p[:], b2S[:], pr[:], OP.subtract)

            # r2y = 2*inv*(scb*y)*y - 2*v*y ; agg_sc = (wv*inv)^T r2y
            nc.vector.tensor_tensor(gt1[:], ua[:], y[:], OP.mult)
            nc.vector.tensor_scalar(gt1[:], gt1[:], scol[:, 2:3], None, OP.mult)
            r2y2 = pk.tile([T, DC], F32, name='r2y2')
            nc.vector.tensor_tensor(r2y2[:], vals[:], y[:], OP.mult)
            nc.vector.tensor_scalar(r2y2[:], r2y2[:], 2.0, None, OP.mult)
            nc.vector.tensor_tensor(gt1[:], gt1[:], r2y2[:], OP.subtract)
            wiv = pk.tile([T, 1], F32, name='wiv')
            nc.vector.tensor_tensor(wiv[:], wvS[:], scol[:, 0:1], OP.mult)
            pr2 = psB.tile([1, DC], F32, name='prow2', tag='acc')
            nc.tensor.matmul(pr2[:], wiv[:],
                             gt1[:], start=True, stop=True)
            scp = pk.tile([1, DC], F32, name='scp')
            nc.vector.tensor_tensor(scp[:], scS[:], pr2[:], OP.subtract)
            scros = pk.tile([1, DC], F32, name='scros')
            nc.vector.tensor_tensor(scros[:], scp[:], rosS[:], OP.mult)

            # transpose rows to column layout [DC, 1]: b2pT, scpT, scrosT
            colrows = pk.tile([CQ, NQ * 3], F32, name='colrows')
            for q in range(NQ):
                for j, row in enumerate((b2p, scp, scros)):
                    pt = psB.tile([CQ, 1], F32, name='crps', tag='mm64')
                    nc.tensor.transpose(pt[:], row[:, q * CQ:(q + 1) * CQ],
                                        idn[0:1, 0:1])
                    nc.vector.tensor_copy(colrows[:, q * 3 + j:q * 3 + j + 1],
                                          pt[:])

            # yfT chunks [98, T] = w2p^T @ hfT + b2'T ; squares and partials
            yfT = pk.tile([CQ, NQ * T], F32, name='yfT')
            sqf = pk.tile([CQ, NQ * T], F32, name='sqf')
            ssqf = pk.tile([CQ, NQ * T], F32, name='ssqf')
            for q in range(NQ):
                pf = psB.tile([CQ, T], F32, name='pyf', tag='acc')
                for m in range(HT):
                    nc.tensor.matmul(pf[:],
                                     w2p[:, m * DC + q * CQ:m * DC + (q + 1) * CQ],
                                     hfT[:, m * T:(m + 1) * T],
                                     start=(m == 0), stop=(m == HT - 1))
                sl = slice(q * T, (q + 1) * T)
                nc.vector.tensor_scalar(yfT[:, sl], pf[:],
                                        colrows[:, q * 3:q * 3 + 1], None,
                                        OP.add)
                nc.vector.tensor_tensor(sqf[:, sl], yfT[:, sl], yfT[:, sl],
                                        OP.mult)
                nc.vector.tensor_scalar(ssqf[:, sl], yfT[:, sl],
                                        colrows[:, q * 3 + 1:q * 3 + 2], None,
                                        OP.mult)
                nc.vector.tensor_tensor(ssqf[:, sl], ssqf[:, sl], ssqf[:, sl],
                                        OP.mult)
            if debug:
                for q in range(NQ):
                    nc.sync.dma_start(d_yfT[q * CQ:(q + 1) * CQ, :],
                                      yfT[:, q * T:(q + 1) * T])
            pf1 = psB.tile([1, T], F32, name='pf1', tag='acc')
            pf2 = psB.tile([1, T], F32, name='pf2', tag='acc')
            for q in range(NQ):
                nc.tensor.matmul(pf1[:], ocol[0:CQ, :],
                                 sqf[:, q * T:(q + 1) * T],
                                 start=(q == 0), stop=(q == NQ - 1))
            for q in range(NQ):
                nc.tensor.matmul(pf2[:], ocol[0:CQ, :],
                                 ssqf[:, q * T:(q + 1) * T],
                                 start=(q == 0), stop=(q == NQ - 1))
            fin0 = pk.tile([1, T], F32, name='fin0')
            fin1 = pk.tile([1, T], F32, name='fin1')
            nc.vector.tensor_copy(fin0[:], pf1[:])
            nc.vector.tensor_copy(fin1[:], pf2[:])

            # ---- R3: AllReduce final scalars ----
            r3i = pd.tile([2, T], F32, name='r3i')
            r3o = pd.tile([2, T], F32, name='r3o')
            nc.gpsimd.dma_start(r3i[0:1, :], fin0[:])
            nc.gpsimd.dma_start(r3i[1:2, :], fin1[:])
            oPre = pk.tile([CQ, NQ * T], F32, name='outsb')
            for q in range(NQ):
                sl = slice(q * T, (q + 1) * T)
                nc.vector.tensor_scalar(oPre[:, sl], yfT[:, sl],
                                        colrows[:, q * 3 + 2:q * 3 + 3], None,
                                        OP.mult)
            nc.gpsimd.collective_compute(
                'AllReduce', OP.add, replica_groups=[list(range(NCORES))],
                ins=[r3i.opt()], outs=[r3o.opt()])

            # invf = rsqrt(Cf/D + eps); invp = rsqrt(invf^2 * Af/D + eps)
            Cft = pk.tile([1, T], F32, name='Cft')
            Aft = pk.tile([1, T], F32, name='Aft')
            nc.sync.dma_start(Cft[:], r3o[0:1, :])
            nc.sync.dma_start(Aft[:], r3o[1:2, :])
            invft = pk.tile([1, T], F32, name='invft')
            invpt = pk.tile([1, T], F32, name='invpt')
            fft = pk.tile([1, T], F32, name='fft')
            nc.scalar.activation(invft[:], Cft[:], AF.Sqrt,
                                 bias=epsT[0:1, :], scale=1.0 / D)
            nc.vector.reciprocal(invft[:], invft[:])
            nc.vector.tensor_tensor(invpt[:], invft[:], invft[:], OP.mult)
            nc.vector.tensor_tensor(invpt[:], invpt[:], Aft[:], OP.mult)
            nc.scalar.activation(invpt[:], invpt[:], AF.Sqrt,
                                 bias=epsT[0:1, :], scale=1.0 / D)
            nc.vector.reciprocal(invpt[:], invpt[:])
            nc.vector.tensor_tensor(fft[:], invft[:], invpt[:], OP.mult)
            ffb = pk.tile([128, T], F32, name='ffb')
            pt = psB.tile([128, T], F32, name='ffps', tag='mm64')
            nc.tensor.matmul(pt[:], o128[:], fft[:], start=True, stop=True)
            nc.vector.tensor_copy(ffb[:], pt[:])

            # out[q*CQ:...] = oPre * ff  (oPre computed before R3 landed)
            for q in range(NQ):
                sl = slice(q * T, (q + 1) * T)
                nc.vector.tensor_tensor(oPre[:, sl], oPre[:, sl],
                                        ffb[0:CQ, :], OP.mult)
                nc.sync.dma_start(out[q * CQ:(q + 1) * CQ, :], oPre[:, sl])

    _split_excess_waits(nc)
    return nc, sorted(dbg_outs.keys())


def make_inputs(inputs):
    """Build the 8 per-core input dicts from the full problem inputs."""
    x = np.asarray(inputs['x'], np.float32)
    x_t = np.transpose(x, (0, 2, 3, 1))
    X72 = _build_im2col(x_t)
    W73 = _build_w73(np.asarray(inputs['conv_k_w'], np.float32),
                     np.asarray(inputs['conv_k_b'], np.float32),
                     np.asarray(inputs['conv_v_w'], np.float32),
                     np.asarray(inputs['conv_v_b'], np.float32))
    perm, valid = _din_perm()
    dkw = np.asarray(inputs['dense_k_w'], np.float32)
    dvw = np.asarray(inputs['dense_v_w'], np.float32)
    Wk_full = np.zeros((DINP, D), np.float32)
    Wv_full = np.zeros((DINP, D), np.float32)
    Wk_full[valid] = dkw[perm[valid]]
    Wv_full[valid] = dvw[perm[valid]]

    w1 = np.asarray(inputs['mem_w1'], np.float32)
    w2 = np.asarray(inputs['mem_w2'], np.float32)
    sc = np.asarray(inputs['mem_scale'], np.float32)
    ros = np.asarray(inputs['rms_out_scale'], np.float32)
    dkb = np.asarray(inputs['dense_k_b'], np.float32)
    dvb = np.asarray(inputs['dense_v_b'], np.float32)
    b1 = np.asarray(inputs['mem_b1'], np.float32)
    b2 = np.asarray(inputs['mem_b2'], np.float32)

    X72c = np.zeros((NCONV, 80, 512), np.float32)
    X72c[:, :73, :] = X72.reshape(73, NCONV, 512).transpose(1, 0, 2)
    X72c = X72c.reshape(NCONV * 80, 512)
    base = {
        'X72': X72c, 'W73': W73,
        'b1row8': (b1 / NCORES).reshape(1, H),
        'rmspk': _rms_pattern(np.asarray(inputs['rms_k_scale'], np.float32)),
        'rmspv': _rms_pattern(np.asarray(inputs['rms_v_scale'], np.float32)),
        'S4': _s4(), 'wv': _wvec().reshape(T, 1),
        'ones1x64': np.ones((1, T), np.float32),
        'ones1x128': np.ones((1, 128), np.float32),
        'onescol': np.ones((128, 1), np.float32),
        'ident': np.eye(128, dtype=np.float32),
    }
    DCP = 512
    in_maps = []
    for c in range(NCORES):
        sl = slice(c * DC, (c + 1) * DC)
        m = dict(base)
        wkp = np.zeros((DINP, DCP), np.float32)
        wkp[:, :DC] = Wk_full[:, sl]
        wvp = np.zeros((DINP, DCP), np.float32)
        wvp[:, :DC] = Wv_full[:, sl]
        m['WkC'] = wkp
        m['WvC'] = wvp
        bkp = np.zeros((1, DCP), np.float32)
        bkp[0, :DC] = dkb[sl]
        bvp = np.zeros((1, DCP), np.float32)
        bvp[0, :DC] = dvb[sl]
        m['bkC'] = bkp
        m['bvC'] = bvp
        w1c = w1[sl, :]
        m['w1C'] = np.ascontiguousarray(
            w1c.reshape(NQ, CQ, H).transpose(1, 0, 2).reshape(CQ, NQ * H))
        w2c = w2[:, sl]
        m['w2C'] = np.ascontiguousarray(
            w2c.reshape(HT, 128, DC).transpose(1, 0, 2).reshape(128, HT * DC))
        m['b2C'] = b2[sl].reshape(1, DC)
        m['scC'] = sc[sl].reshape(1, DC)
        m['rosC'] = ros[sl].reshape(1, DC)
        m['scsqT'] = np.ascontiguousarray(
            (sc[sl] ** 2).reshape(NQ, CQ).T)
        in_maps.append(m)
    return in_maps


def kernel(**inputs):
    if 'nc' not in _NC_CACHE:
        _NC_CACHE['nc'], _ = build_nc(debug=False)
    nc = _NC_CACHE['nc']
    in_maps = make_inputs(inputs)
    res = run_bass_kernel_spmd(nc, in_maps, list(range(NCORES)))
    YT = np.concatenate([res.results[c]['out'] for c in range(NCORES)], axis=0)
    return np.ascontiguousarray(YT.T).reshape(T, 4, 28, 28)

